# revision 1
# baseline (speedup 1.0000x reference)
"""Contrastive-loss kernel for TRN2, data-parallel over batch (8 cores).

Per core: one image. features[b] [256,128,256] -> bilinear-resize(32,64)
(= avg of the 2x2 block at rows/cols 4k+1,4k+2 -- the bilinear weights are
exactly 0.5 at this scale), L2-normalize over C, cos-sim matrix N x N
(N=2048), softmax row-LSE, supervised-contrastive loss.

Math used here (exact rewrite of the reference):
  ll_i/den_i = LSE_i - PS_i/(tau*cnt[lab_i])   for non-ignored i, else 0
  LSE_i = log(sum_m exp(cos(i,m)/tau))         (unmasked logits, all m)
  PS_i  = sum_{m: lab_m == lab_i} cos(i, m)
  loss_b = sum_i(ll_i/den_i) / max(#non-ignored, 1);  loss = mean_b loss_b
The normalize's 0.25 resize factor cancels in cos.  NaN/den=0 handling in
the reference reduces to zeroing ignored rows (a non-ignored row always
matches itself, so den>0 exactly when the row is non-ignored).
"""

import os

import numpy as np

import concourse.bass as bass
import concourse.tile as tile
from concourse import bacc, mybir, bass_utils

F32 = mybir.dt.float32
F32R = mybir.dt.float32r
BF16 = mybir.dt.bfloat16
I32 = mybir.dt.int32
AF = mybir.ActivationFunctionType
OP = mybir.AluOpType

B, C, H, W = 8, 256, 128, 256
OUT_H, OUT_W = 32, 64
N = OUT_H * OUT_W  # 2048
TAU = 0.07
IGNORE = 19.0
N_CORES = 8
NT = N // 128  # 16 row tiles of 128 pixels
NCH = 4  # pixel chunks of 512 (= 8 output rows each)
CW = 512  # chunk width (pixels)
HC = 8  # output rows per chunk
NG = 2  # channel groups of 128

_CACHE = {}


def _build(phase: int = 4):
    nc = bacc.Bacc("TRN2", target_bir_lowering=False, num_devices=N_CORES)
    feat = nc.dram_tensor("feat", [C, H, W], F32, kind="ExternalInput").ap()
    lab = nc.dram_tensor("lab", [H, W], I32, kind="ExternalInput").ap()
    iota = nc.dram_tensor("iota", [128, 20], F32, kind="ExternalInput").ap()
    ident = nc.dram_tensor("ident", [128, 128], F32, kind="ExternalInput").ap()
    ones = nc.dram_tensor("ones", [128, 128], F32, kind="ExternalInput").ap()
    out = nc.dram_tensor("out", [2], F32, kind="ExternalOutput").ap()

    with tile.TileContext(nc) as tc:
        with (
            tc.tile_pool(name="xin", bufs=3) as xpool,
            tc.tile_pool(name="rw", bufs=3) as rwpool,
            tc.tile_pool(name="f2", bufs=3) as f2pool,
            tc.tile_pool(name="esc", bufs=4) as escpool,
            tc.tile_pool(name="persist", bufs=1) as pp,
            tc.tile_pool(name="small", bufs=4) as sm,
        ):
            ps_cm = tc.tile_pool(name="psmall", bufs=4, space="PSUM")
            ps = ps_cm.__enter__()
            # ---- constants ----
            iota_s = pp.tile([128, 20], F32, tag="iota")
            nc.sync.dma_start(out=iota_s, in_=iota)
            ident_s = pp.tile([128, 128], F32, tag="ident")
            nc.sync.dma_start(out=ident_s, in_=ident)
            ones_s = pp.tile([128, 128], F32, tag="ones")
            nc.sync.dma_start(out=ones_s, in_=ones)
            ident_r = pp.tile([128, 128], BF16, tag="identr")
            nc.vector.tensor_copy(out=ident_r, in_=ident_s)
            ones_r = pp.tile([128, 128], BF16, tag="onesr")
            nc.vector.tensor_copy(out=ones_r, in_=ones_s)

            # ---- labels: L[p, t] = lab[4h, 4w], pixel n = 128*t + p ----
            # n = h*64 + w  =>  p = 64*(h%2) + w, t = h//2
            lab_i = pp.tile([128, NT], I32, tag="labi")
            for dh in range(2):
                src = bass.AP(
                    tensor=lab.tensor,
                    offset=lab.offset + dh * 4 * W,
                    ap=[[4, 64], [8 * W, NT]],
                )
                nc.sync.dma_start(out=lab_i[64 * dh : 64 * (dh + 1), :], in_=src)
            lab_f = pp.tile([128, NT], F32, tag="labf")
            nc.vector.tensor_copy(out=lab_f, in_=lab_i)
            # onehot per m-tile: oh[p, t, c] = (lab[128t+p] == c)
            oh = pp.tile([128, NT, 20], F32, tag="oh")
            for t in range(NT):
                nc.vector.tensor_scalar(
                    out=oh[:, t, :],
                    in0=iota_s,
                    scalar1=lab_f[:, t : t + 1],
                    scalar2=None,
                    op0=OP.is_equal,
                )
            oh_r = pp.tile([128, NT, 20], BF16, tag="ohr")
            nc.vector.tensor_copy(out=oh_r, in_=oh)
            ni = pp.tile([128, NT], F32, tag="ni")  # 1 - ignore mask
            nc.vector.tensor_scalar(
                out=ni, in0=lab_f, scalar1=IGNORE, scalar2=None, op0=OP.not_equal
            )

            # ---- class counts -> broadcast [128, 20] ----
            psc = ps.tile([20, 2], F32, tag="ps")
            for t in range(NT):
                nc.tensor.matmul(
                    psc,
                    oh_r[:, t, :],
                    ones_r[:, 0:2],
                    start=(t == 0),
                    stop=(t == NT - 1),
                )
            cnt_col = sm.tile([20, 1], F32, tag="cntc")
            nc.vector.tensor_copy(out=cnt_col, in_=psc[:, 0:1])
            psr = ps.tile([1, 20], F32, tag="ps")
            nc.tensor.transpose(psr, cnt_col, ident_s[0:20, 0:20])
            cnt_row = sm.tile([1, 20], F32, tag="cntr")
            nc.vector.tensor_copy(out=cnt_row, in_=psr)
            pscb = ps.tile([128, 20], F32, tag="ps")
            nc.tensor.matmul(pscb, ones_s[0:1, :], cnt_row, start=True, stop=True)
            cb = pp.tile([128, 20], F32, tag="cb")
            nc.vector.tensor_copy(out=cb, in_=pscb)

            # ---- feature load + resize + normalize, chunk by chunk ----
            favg = [pp.tile([128, N], F32, tag=f"favg{g}", name=f"favg{g}") for g in range(NG)]
            favgn = [pp.tile([128, N], BF16, tag=f"favgn{g}", name=f"favgn{g}") for g in range(NG)]
            favgnt = [
                pp.tile([128, NT, 128], BF16, tag=f"favgnt{g}", name=f"favgnt{g}")
                for g in range(NG)
            ]
            rnb = pp.tile([128, N], F32, tag="rnb")
            eps1 = pp.tile([1, 1], F32, tag="eps1")
            nc.vector.memset(eps1, 1e-24)
            rn_row = pp.tile([1, N], F32, tag="rnrow")
            lns = pp.tile([1, N], F32, tag="lns")

            for cidx in range(NCH if phase >= 2 else 0):
                f2s = []
                for g in range(NG):
                    xc = xpool.tile([128, HC, 512], F32, tag="xc")
                    # rows 4h+1, 4h+2 are adjacent: one 512-elem run per (c, h)
                    src = bass.AP(
                        tensor=feat.tensor,
                        offset=feat.offset
                        + g * 128 * H * W
                        + (4 * (cidx * HC) + 1) * W,
                        ap=[[H * W, 128], [4 * W, HC], [1, 512]],
                    )
                    nc.sync.dma_start(out=xc, in_=src)
                    # W-pairs first (1024 out elems), then H-pairs (512)
                    x4 = xc.rearrange("p h (r c) -> p h r c", r=2)
                    rw = rwpool.tile([128, HC, 2, OUT_W], F32, tag="rw")
                    nc.vector.tensor_add(
                        rw, x4[:, :, :, 1:254:4], x4[:, :, :, 2:255:4]
                    )
                    fv = favg[g].rearrange("p (h w) -> p h w", w=OUT_W)
                    nc.vector.tensor_add(
                        fv[:, cidx * HC : (cidx + 1) * HC, :],
                        rw[:, :, 0, :],
                        rw[:, :, 1, :],
                    )
                    f2 = f2pool.tile([128, CW], F32, tag="f2")
                    cs = slice(cidx * CW, (cidx + 1) * CW)
                    nc.vector.tensor_mul(f2, favg[g][:, cs], favg[g][:, cs])
                    f2s.append(f2)
                cs = slice(cidx * CW, (cidx + 1) * CW)
                # sq-norms: ones^T @ favg^2, accumulated over channel groups
                psn = ps.tile([1, CW], F32, tag="ps")
                for g in range(NG):
                    nc.tensor.matmul(
                        psn, ones_s[:, 0:1], f2s[g], start=(g == 0), stop=(g == NG - 1)
                    )
                # rn = s^-1/2 = exp(-0.5*ln(s)) (Ln/Exp share one ACT table set)
                nc.scalar.activation(
                    out=lns[:, cs], in_=psn, func=AF.Ln, bias=eps1, scale=1.0
                )
                nc.scalar.activation(
                    out=rn_row[:, cs], in_=lns[:, cs], func=AF.Exp, bias=0.0, scale=-0.5
                )
                # broadcast rn to all partitions via K=1 matmul of ones-row
                psb = ps.tile([128, CW], F32, tag="ps")
                nc.tensor.matmul(
                    psb, ones_s[0:1, :], rn_row[:, cs], start=True, stop=True
                )
                nc.vector.tensor_copy(out=rnb[:, cs], in_=psb)
                for g in range(NG):
                    nc.vector.tensor_mul(favgn[g][:, cs], favg[g][:, cs], rnb[:, cs])
                    for tt in range(4):
                        t = cidx * 4 + tt
                        pst = ps.tile([128, 128], BF16, tag="ps")
                        nc.tensor.transpose(
                            pst, favgn[g][:, t * 128 : (t + 1) * 128], ident_r
                        )
                        nc.vector.tensor_copy(out=favgnt[g][:, t, :], in_=pst)

            # ---- H[ch, c] = sum_m favgn[ch, m] * onehot[m, c] ----
            hs = pp.tile([128, NG, 20], BF16, tag="hs")
            for g in range(NG if phase >= 3 else 0):
                psh = ps.tile([128, 20], F32, tag="ps")
                for t in range(NT):
                    nc.tensor.matmul(
                        psh,
                        favgnt[g][:, t, :],
                        oh_r[:, t, :],
                        start=(t == 0),
                        stop=(t == NT - 1),
                    )
                nc.vector.tensor_copy(out=hs[:, g, :], in_=psh)

            # ---- per-row-tile: perclass gather, den gather ----
            psa = pp.tile([128, NT], F32, tag="psa")  # PS_i
            den = pp.tile([128, NT], F32, tag="den")  # cnt[lab_i]
            scr = sm.tile([128, 20], F32, tag="scr")
            for t in range(NT if phase >= 3 else 0):
                psp = ps.tile([128, 20], F32, tag="ps")
                for g in range(NG):
                    nc.tensor.matmul(
                        psp,
                        favgn[g][:, t * 128 : (t + 1) * 128],
                        hs[:, g, :],
                        start=(g == 0),
                        stop=(g == NG - 1),
                    )
                scr = sm.tile([128, 20], F32, tag="scr")
                nc.vector.tensor_mul(scr, psp, oh[:, t, :])
                nc.vector.tensor_reduce(
                    out=psa[:, t : t + 1],
                    in_=scr,
                    axis=mybir.AxisListType.X,
                    op=OP.add,
                )
                scr2 = sm.tile([128, 20], F32, tag="scr")
                nc.vector.tensor_mul(scr2, oh[:, t, :], cb)
                nc.vector.tensor_reduce(
                    out=den[:, t : t + 1],
                    in_=scr2,
                    axis=mybir.AxisListType.X,
                    op=OP.add,
                )

            ps_cm.__exit__(None, None, None)
            pg_cm = tc.tile_pool(name="pgram", bufs=4, space="PSUM")
            pg = pg_cm.__enter__()

            # ---- Gram half-rows [128,1024] + fused exp/rowsum ----
            # half h of row-tile t covers j-chunks {2h, 2h+1}; available once
            # favgn chunks max(t//4, 2h+1) are ready.
            se = pp.tile([128, NT, 2], F32, tag="se")
            order = sorted(
                ((max(t // 4, 2 * h + 1), t, h) for t in range(NT) for h in range(2))
            )
            for _, t, h in order if phase >= 4 else []:
                psg = pg.tile([128, 2 * CW], F32, tag="pg")
                for g in range(NG):
                    for jj in range(2):
                        j = 2 * h + jj
                        nc.tensor.matmul(
                            psg[:, jj * CW : (jj + 1) * CW],
                            favgn[g][:, t * 128 : (t + 1) * 128],
                            favgn[g][:, j * CW : (j + 1) * CW],
                            start=(g == 0),
                            stop=(g == NG - 1),
                        )
                esc = escpool.tile([128, 2 * CW], F32, tag="esc")
                nc.scalar.activation(
                    out=esc,
                    in_=psg,
                    func=AF.Exp,
                    bias=0.0,
                    scale=1.0 / TAU,
                    accum_out=se[:, t, h : h + 1],
                )

            # ---- finals ----
            if phase < 4:
                dummy = sm.tile([2, 1], F32, tag="dummy")
                nc.vector.memset(dummy, 1.0)
                nc.sync.dma_start(out=out, in_=dummy)
                pg_cm.__exit__(None, None, None)
            else:
                sumexp = sm.tile([128, NT], F32, tag="sumexp")
                nc.vector.tensor_reduce(
                    out=sumexp, in_=se, axis=mybir.AxisListType.X, op=OP.add
                )
                lse = sm.tile([128, NT], F32, tag="lse")
                nc.scalar.activation(
                    out=lse, in_=sumexp, func=AF.Ln, bias=0.0, scale=1.0
                )
                rc = sm.tile([128, NT], F32, tag="rc")
                nc.vector.reciprocal(out=rc, in_=den)
                t1 = sm.tile([128, NT], F32, tag="t1")
                nc.vector.tensor_mul(t1, psa, rc)
                z0 = sm.tile([128, NT], F32, tag="z0")
                nc.vector.scalar_tensor_tensor(
                    out=z0, in0=t1, scalar=-1.0 / TAU, in1=lse, op0=OP.mult, op1=OP.add
                )
                pk = sm.tile([128, 2], F32, tag="pk")
                zf = sm.tile([128, NT], F32, tag="zf")
                nc.vector.tensor_mul(zf, z0, ni)
                nc.vector.tensor_reduce(
                    out=pk[:, 0:1], in_=zf, axis=mybir.AxisListType.X, op=OP.add
                )
                nc.vector.tensor_reduce(
                    out=pk[:, 1:2], in_=ni, axis=mybir.AxisListType.X, op=OP.add
                )
                psf = pg.tile([2, 1], F32, tag="pg")
                nc.tensor.matmul(psf, pk, ones_s[:, 0:1], start=True, stop=True)
                osb = sm.tile([2, 1], F32, tag="osb")
                nc.vector.tensor_copy(out=osb, in_=psf)
                nc.sync.dma_start(out=out, in_=osb)
            pg_cm.__exit__(None, None, None)

    nc.compile()
    return nc


def kernel(features: np.ndarray, labels: np.ndarray) -> np.ndarray:
    if "nc" not in _CACHE:
        _CACHE["nc"] = _build(int(os.environ.get("KERNEL_PHASE", "4")))
    nc = _CACHE["nc"]

    iota = np.tile(np.arange(20, dtype=np.float32), (128, 1))
    ident = np.eye(128, dtype=np.float32)
    ones = np.ones((128, 128), dtype=np.float32)
    in_maps = [
        {
            "feat": np.ascontiguousarray(features[b]),
            "lab": np.ascontiguousarray(labels[b]),
            "iota": iota,
            "ident": ident,
            "ones": ones,
        }
        for b in range(B)
    ]
    trace = bool(int(os.environ.get("KERNEL_TRACE", "0")))
    res = bass_utils.run_bass_kernel_spmd(
        nc, in_maps, core_ids=list(range(N_CORES)), trace=trace
    )
    if trace and res.exec_time_ns is not None:
        print(f"HW exec time: {res.exec_time_ns} ns")
        _CACHE["exec_time_ns"] = res.exec_time_ns
        _CACHE["trace"] = res.instructions_and_trace
    losses = []
    for b in range(B):
        z, cnt = res.results[b]["out"]
        losses.append(z / max(cnt, 1.0))
    return np.float32(np.mean(np.float32(losses)))

